# revision 9
# baseline (speedup 1.0000x reference)
"""Multi-head attention (B=4, N=2048, C=768, H=12) on 8 trn2 NeuronCores.

Sharding: core c handles batch b = c//2 and heads hh = c%2 (6 heads each:
global heads 6*hh .. 6*hh+5). Each core computes Q/K/V for its 6 heads over
all 2048 queries/keys, full attention for those heads, and a PARTIAL output
projection (contraction over its 384 channels). The host sums the two
partial projections per batch and adds the bias.

Per-core on-chip layout (all bf16 on the PE):
  xtb  = x_b.T          6 x [128, 2048]
  ktb  = Wk_h @ x.T     3 x [128, 2048]   tile t rows 64*(h%2) hold head h=2t+..
  qtb  = Wq_h @ x.T     3 x [128, 2048]
  vtb  = x @ Wv_h.T    16 x [128, 390]    per key-tile; head h cols 65h..65h+63,
                                          col 65h+64 == 1.0 (ones column makes
                                          the PV matmul also emit softmax
                                          denominators)
  st   = K_h^T Q_h      [128k, 1024q] PSUM; exp on ScalarE (scale folded)
  ot   = [V_h|1]^T P    [65, 1024] PSUM accumulated over 16 key tiles
  otb  = normalized out 3 x [128, 2048] bf16 (head-major channel layout)
  y    = otb.T @ WpT_h  [2048, 768] fp32 partial, summed on host

The scalar engine (exp: 192 x [128,1024] activations) is the pacing floor;
QKV/proj matmuls are interleaved into the attention PE stream via a backlog
queue so the PE never idles long enough to trip the HAM clock-gate.
"""

import os
import sys
from collections import deque

import numpy as np
import ml_dtypes

sys.path.insert(0, "/opt/trn_rl_repo")

import concourse.bass as bass
from concourse import bacc
import concourse.mybir as mybir
from concourse.tile import TileContext
from concourse.bass_utils import run_bass_kernel_spmd
from concourse.dma_utils import dma_copy

P = 128
C = 768
CH = 384             # channels per core (6 heads)
NK = 2048
NQ = 2048
QC = 1024            # query chunk (exp instruction free size)
NH = 6               # local heads
DH = 64
CT = C // P          # 6 contraction tiles for QKV linears
PT = CH // P         # 3 contraction tiles for proj
KT = NK // P         # 16 key tiles
SCALE = DH ** -0.5
F32 = mybir.dt.float32
BF16 = mybir.dt.bfloat16
BF16_NP = ml_dtypes.bfloat16

LAST_RESULT = None
_PROG = None


def _build_program() -> bass.Bass:
    nc = bacc.Bacc(None, target_bir_lowering=False)

    xt = nc.dram_tensor("xt", [C, NK], BF16, kind="ExternalInput")
    wkt = nc.dram_tensor("wkt", [C, CH], BF16, kind="ExternalInput")
    wqt = nc.dram_tensor("wqt", [C, CH], BF16, kind="ExternalInput")
    wvt = nc.dram_tensor("wvt", [C, CH], BF16, kind="ExternalInput")
    wpt = nc.dram_tensor("wpt", [CH, C], BF16, kind="ExternalInput")
    y = nc.dram_tensor("y", [NQ, C], F32, kind="ExternalOutput")
    debug = bool(os.environ.get("BASS_DEBUG_DUMP"))
    if debug:
        dkt = nc.dram_tensor("dkt", [CH, NK], F32, kind="ExternalOutput")
        dqt = nc.dram_tensor("dqt", [CH, NQ], F32, kind="ExternalOutput")
        dvt = nc.dram_tensor("dvt", [KT * P, NH * (DH + 1)], F32, kind="ExternalOutput")
        dot = nc.dram_tensor("dot", [CH, NQ], F32, kind="ExternalOutput")

    with TileContext(nc) as tc:
        with (
            tc.tile_pool(name="persist", bufs=1) as persist,
            tc.tile_pool(name="pP", bufs=3) as pP,
            tc.tile_pool(name="norm", bufs=2) as nsb,
            tc.tile_pool(name="ysb", bufs=2) as ysb,
            tc.tile_pool(name="psa", bufs=2, space="PSUM") as psa,
            tc.tile_pool(name="psb", bufs=2, space="PSUM") as psb,
        ):
            # ---- DMA inputs (weights for K/Q first so attention starts early)
            def load(dram, rows, cols, tag):
                tiles = []
                for i in range(rows // P):
                    t = persist.tile([P, cols], BF16, tag=f"{tag}{i}")
                    dma_copy(nc.gpsimd, t[:, :], dram[i * P:(i + 1) * P, :])
                    tiles.append(t)
                return tiles

            wkb = load(wkt, C, CH, "wk")
            wqb = load(wqt, C, CH, "wq")
            # x in query-chunk-major order so the first K/Q column chunks can
            # start as soon as possible
            xtb = [persist.tile([P, NK], BF16, tag=f"xt{i}", name=f"xt{i}")
                   for i in range(CT)]
            for qch in range(4):
                for i in range(CT):
                    dma_copy(nc.gpsimd, xtb[i][:, qch * 512:(qch + 1) * 512],
                             xt[i * P:(i + 1) * P, qch * 512:(qch + 1) * 512])
            wvb = load(wvt, C, CH, "wv")
            wpb = load(wpt, CH, C, "wp")

            onesb = persist.tile([1, DH], BF16, tag="ones")
            nc.gpsimd.memset(onesb[:, :], 1.0)

            ktb = [persist.tile([P, NK], BF16, tag=f"kt{i}", name=f"kt{i}")
                   for i in range(PT)]
            qtb = [persist.tile([P, NQ], BF16, tag=f"qt{i}", name=f"qt{i}")
                   for i in range(PT)]
            vtb = [persist.tile([P, NH * (DH + 1)], BF16, tag=f"v{i}", name=f"v{i}")
                   for i in range(KT)]
            for i in range(KT):
                ones_cols = vtb[i][:, :].rearrange(
                    "p (h e) -> p h e", e=DH + 1)[:, :, DH:DH + 1]
                nc.gpsimd.memset(ones_cols, 1.0)
            otb = [persist.tile([P, NQ], BF16, tag=f"ot{i}", name=f"ot{i}")
                   for i in range(PT)]

            # ---- work units (emitted inline or pumped from the backlog) ----
            def kq_unit(wtiles, out_tiles, pair, qch):
                # one [128, 512] column chunk of K^T (or Q^T) for head pair
                ps = psa.tile([P, QC], F32, tag="a")
                for k in range(CT):
                    nc.tensor.matmul(
                        ps[:, 0:512],
                        lhsT=wtiles[k][:, pair * P:(pair + 1) * P],
                        rhs=xtb[k][:, qch * 512:(qch + 1) * 512],
                        start=(k == 0), stop=(k == CT - 1),
                    )
                nc.vector.tensor_copy(
                    out_tiles[pair][:, qch * 512:(qch + 1) * 512], ps[:, 0:512]
                )

            def v_unit(kt):
                # V rows for key tile kt: [128, 384], scattered into stride-65
                # columns of vtb[kt] (leaves the per-head ones column alone)
                ps = psa.tile([P, QC], F32, tag="a")
                for k in range(CT):
                    nc.tensor.matmul(
                        ps[:, 0:CH],
                        lhsT=xtb[k][:, kt * P:(kt + 1) * P],
                        rhs=wvb[k][:, :],
                        start=(k == 0), stop=(k == CT - 1),
                    )
                dst = vtb[kt][:, :].rearrange("p (h e) -> p h e", e=DH + 1)[:, :, 0:DH]
                src = ps[:, 0:CH].rearrange("p (h e) -> p h e", e=DH)
                nc.vector.tensor_copy(dst, src)

            def proj_unit(qc, qt):
                # partial projection rows [128, 768] in two free-dim chunks
                q0 = qc * QC + qt * P
                yt = ysb.tile([P, C], F32, tag="y")
                for c0, csz in ((0, 512), (512, C - 512)):
                    ps = psa.tile([P, QC], F32, tag="a")
                    for k in range(PT):
                        nc.tensor.matmul(
                            ps[:, 0:csz],
                            lhsT=otb[k][:, q0:q0 + P],
                            rhs=wpb[k][:, c0:c0 + csz],
                            start=(k == 0), stop=(k == PT - 1),
                        )
                    nc.vector.tensor_copy(yt[:, c0:c0 + csz], ps[:, 0:csz])
                nc.sync.dma_start(out=y[q0:q0 + P, :], in_=yt[:, :])

            backlog = deque()

            def pump(n):
                for _ in range(min(n, len(backlog))):
                    backlog.popleft()()

            # Minimal prelude: just enough K/Q/V for (h0, qc0) to start; the
            # rest feeds the attention-loop backlog in dependency order.
            kq_unit(wkb, ktb, 0, 0)
            kq_unit(wqb, qtb, 0, 0)
            kq_unit(wqb, qtb, 0, 1)
            v_unit(0)
            v_unit(1)
            for qch in (1, 2, 3):
                backlog.append(lambda q=qch: kq_unit(wkb, ktb, 0, q))
            for kt in range(2, KT):
                backlog.append(lambda kt=kt: v_unit(kt))
            for pair in (1, 2):
                for qch in range(4):
                    backlog.append(
                        lambda p=pair, q=qch: kq_unit(wkb, ktb, p, q))
                for qch in range(4):
                    backlog.append(
                        lambda p=pair, q=qch: kq_unit(wqb, qtb, p, q))
            backlog.append(lambda: kq_unit(wqb, qtb, 0, 2))
            backlog.append(lambda: kq_unit(wqb, qtb, 0, 3))

            # ---- attention ----
            def attention(h, qc, pump_every=3):
                pair, hr = h // 2, (h % 2) * DH
                ot = psb.tile([DH + 1, QC], F32, tag="b")

                def av(pt, kt):
                    for j in range(2):
                        nc.tensor.matmul(
                            ot[:, j * 512:(j + 1) * 512],
                            lhsT=vtb[kt][:, h * (DH + 1):(h + 1) * (DH + 1)],
                            rhs=pt[:, j * 512:(j + 1) * 512],
                            start=(kt == 0), stop=(kt == KT - 1),
                        )

                pending = None
                for kt in range(KT):
                    st = psa.tile([P, QC], F32, tag="a")
                    for j in range(2):
                        nc.tensor.matmul(
                            st[:, j * 512:(j + 1) * 512],
                            lhsT=ktb[pair][hr:hr + DH, kt * P:(kt + 1) * P],
                            rhs=qtb[pair][hr:hr + DH,
                                          qc * QC + j * 512:qc * QC + (j + 1) * 512],
                            start=True, stop=True,
                            tile_position=(hr, 0),
                        )
                    if kt % pump_every == 0:
                        pump(1)
                    if pending is not None:
                        av(*pending)
                    pt = pP.tile([P, QC], BF16, tag="p")
                    nc.scalar.activation(
                        pt[:, :], st[:, :],
                        mybir.ActivationFunctionType.Exp, scale=SCALE,
                    )
                    pending = (pt, kt)
                pump(1)
                av(*pending)

                # stash the unnormalized output + denominators (fast PSUM
                # drain) and defer the reciprocal/broadcast/multiply into the
                # backlog so the next head's matmuls start immediately
                osb = nsb.tile([DH, QC], F32, tag="osb")
                den = nsb.tile([1, QC], F32, tag="den")
                nc.vector.tensor_copy(osb[:, :], ot[0:DH, :])
                nc.vector.tensor_copy(den[:, :], ot[DH:DH + 1, :])

                def finish_norm(pair=pair, hr=hr, qc=qc, osb=osb, den=den):
                    rec = nsb.tile([1, QC], F32, tag="rec")
                    nc.vector.reciprocal_approx_fast(out=rec[:, :], in_=den[:, :])
                    recb = nsb.tile([1, QC], BF16, tag="recb")
                    nc.vector.tensor_copy(recb[:, :], rec[:, :])
                    rb = psa.tile([P, QC], F32, tag="a")
                    for j in range(2):
                        nc.tensor.matmul(
                            rb[0:DH, j * 512:(j + 1) * 512],
                            lhsT=onesb[0:1, :],
                            rhs=recb[0:1, j * 512:(j + 1) * 512],
                            start=True, stop=True,
                        )
                    nc.vector.tensor_mul(
                        otb[pair][hr:hr + DH, qc * QC:(qc + 1) * QC],
                        osb[:, :], rb[0:DH, :],
                    )
                return finish_norm

            norm_pending = []

            def once(fn):
                state = {"done": False}

                def g():
                    if not state["done"]:
                        state["done"] = True
                        fn()
                return g

            def flush_norms():
                # osb/den staging has 2 buffers; never leave more than one
                # unfinished norm outstanding
                while norm_pending:
                    norm_pending.pop(0)()

            for qc in range(2):
                for h in range(NH):
                    flush_norms()
                    pe = 1 if (qc == 0 and h < 2) else 3
                    fn = once(attention(h, qc, pump_every=pe))
                    backlog.appendleft(fn)
                    norm_pending.append(fn)
                if qc == 0:
                    flush_norms()
                    for qt in range(QC // P):
                        backlog.append(lambda qt=qt: proj_unit(0, qt))
            flush_norms()
            pump(len(backlog))
            for qt in range(QC // P):
                proj_unit(1, qt)
            if debug:
                for i in range(PT):
                    tmp = ysb.tile([P, NK], F32, tag="dbg")
                    nc.vector.tensor_copy(tmp[:, :], ktb[i][:, :])
                    nc.sync.dma_start(out=dkt[i * P:(i + 1) * P, :], in_=tmp[:, :])
                    tmp = ysb.tile([P, NQ], F32, tag="dbg")
                    nc.vector.tensor_copy(tmp[:, :], qtb[i][:, :])
                    nc.sync.dma_start(out=dqt[i * P:(i + 1) * P, :], in_=tmp[:, :])
                    tmp = ysb.tile([P, NQ], F32, tag="dbg")
                    nc.vector.tensor_copy(tmp[:, :], otb[i][:, :])
                    nc.sync.dma_start(out=dot[i * P:(i + 1) * P, :], in_=tmp[:, :])
                for i in range(KT):
                    tmp = ysb.tile([P, NH * (DH + 1)], F32, tag="dbgv")
                    nc.vector.tensor_copy(tmp[:, :], vtb[i][:, :])
                    nc.sync.dma_start(out=dvt[i * P:(i + 1) * P, :], in_=tmp[:, :])

    nc.compile()
    return nc


def _get_prog() -> bass.Bass:
    global _PROG
    if _PROG is None:
        _PROG = _build_program()
    return _PROG


def kernel(x, Wq, Wk, Wv, Wp, bp):
    global LAST_RESULT
    x = np.asarray(x, np.float32)
    Wq = np.asarray(Wq, np.float32)
    Wk = np.asarray(Wk, np.float32)
    Wv = np.asarray(Wv, np.float32)
    Wp = np.asarray(Wp, np.float32)
    bp = np.asarray(bp, np.float32)

    B, N, _ = x.shape
    xts = [np.ascontiguousarray(x[b].T).astype(BF16_NP) for b in range(B)]
    wk_h, wq_h, wv_h, wp_h = [], [], [], []
    for hh in range(2):
        r = slice(hh * CH, (hh + 1) * CH)
        wq_h.append(np.ascontiguousarray(Wq[r].T).astype(BF16_NP))
        wk_h.append(np.ascontiguousarray(Wk[r].T).astype(BF16_NP))
        wv_h.append(np.ascontiguousarray(Wv[r].T).astype(BF16_NP))
        wp_h.append(np.ascontiguousarray(Wp.T[r]).astype(BF16_NP))

    in_maps = []
    for core in range(8):
        b, hh = core // 2, core % 2
        in_maps.append({
            "xt": xts[b],
            "wkt": wk_h[hh], "wqt": wq_h[hh],
            "wvt": wv_h[hh], "wpt": wp_h[hh],
        })

    res = run_bass_kernel_spmd(
        _get_prog(), in_maps, core_ids=list(range(8)),
        trace=bool(os.environ.get("BASS_TRACE")),
    )
    LAST_RESULT = res

    out = np.empty((B, N, C), np.float32)
    for b in range(B):
        out[b] = res.results[2 * b]["y"] + res.results[2 * b + 1]["y"] + bp
    return out


# revision 10
# speedup vs baseline: 1.4623x; 1.4623x over previous
"""Multi-head attention (B=4, N=2048, C=768, H=12) on 8 trn2 NeuronCores.

Sharding: core c handles batch b = c//2 and heads hh = c%2 (6 heads each:
global heads 6*hh .. 6*hh+5). Each core computes Q/K/V for its 6 heads over
all 2048 queries/keys, full attention for those heads, and a PARTIAL output
projection (contraction over its 384 channels). The host sums the two
partial projections per batch and adds the bias.

Per-core on-chip layout (all bf16 on the PE):
  xtb  = x_b.T          6 x [128, 2048]
  ktb  = Wk_h @ x.T     3 x [128, 2048]   tile t rows 64*(h%2) hold head h=2t+..
  qtb  = Wq_h @ x.T     3 x [128, 2048]
  vtb  = x @ Wv_h.T    16 x [128, 390]    per key-tile; head h cols 65h..65h+63,
                                          col 65h+64 == 1.0 (ones column makes
                                          the PV matmul also emit softmax
                                          denominators)
  st   = K_h^T Q_h      [128k, 1024q] PSUM; exp on ScalarE (scale folded)
  ot   = [V_h|1]^T P    [65, 1024] PSUM accumulated over 16 key tiles
  otb  = normalized out 3 x [128, 2048] bf16 (head-major channel layout)
  y    = otb.T @ WpT_h  [2048, 768] fp32 partial, summed on host

The scalar engine (exp: 192 x [128,1024] activations) is the pacing floor;
QKV/proj matmuls are interleaved into the attention PE stream via a backlog
queue so the PE never idles long enough to trip the HAM clock-gate.
"""

import os
import sys
from collections import deque

import numpy as np
import ml_dtypes

sys.path.insert(0, "/opt/trn_rl_repo")

import concourse.bass as bass
from concourse import bacc
import concourse.mybir as mybir
from concourse.tile import TileContext
from concourse.bass_utils import run_bass_kernel_spmd
from concourse.dma_utils import dma_copy

P = 128
C = 768
CH = 384             # channels per core (6 heads)
NK = 2048
NQ = 2048
QC = 1024            # query chunk (exp instruction free size)
NH = 6               # local heads
DH = 64
CT = C // P          # 6 contraction tiles for QKV linears
PT = CH // P         # 3 contraction tiles for proj
KT = NK // P         # 16 key tiles
SCALE = DH ** -0.5
F32 = mybir.dt.float32
BF16 = mybir.dt.bfloat16
BF16_NP = ml_dtypes.bfloat16

LAST_RESULT = None
_PROG = None


def _build_program() -> bass.Bass:
    nc = bacc.Bacc(None, target_bir_lowering=False)

    xt = nc.dram_tensor("xt", [C, NK], BF16, kind="ExternalInput")
    wkt = nc.dram_tensor("wkt", [C, CH], BF16, kind="ExternalInput")
    wqt = nc.dram_tensor("wqt", [C, CH], BF16, kind="ExternalInput")
    wvt = nc.dram_tensor("wvt", [C, CH], BF16, kind="ExternalInput")
    wpt = nc.dram_tensor("wpt", [CH, C], BF16, kind="ExternalInput")
    y = nc.dram_tensor("y", [NQ, C], F32, kind="ExternalOutput")
    debug = bool(os.environ.get("BASS_DEBUG_DUMP"))
    if debug:
        dkt = nc.dram_tensor("dkt", [CH, NK], F32, kind="ExternalOutput")
        dqt = nc.dram_tensor("dqt", [CH, NQ], F32, kind="ExternalOutput")
        dvt = nc.dram_tensor("dvt", [KT * P, NH * (DH + 1)], F32, kind="ExternalOutput")
        dot = nc.dram_tensor("dot", [CH, NQ], F32, kind="ExternalOutput")

    with TileContext(nc) as tc:
        with (
            tc.tile_pool(name="persist", bufs=1) as persist,
            tc.tile_pool(name="pP", bufs=3) as pP,
            tc.tile_pool(name="norm", bufs=2) as nsb,
            tc.tile_pool(name="ysb", bufs=2) as ysb,
            tc.tile_pool(name="psa", bufs=2, space="PSUM") as psa,
            tc.tile_pool(name="psb", bufs=2, space="PSUM") as psb,
        ):
            # ---- DMA inputs (weights for K/Q first so attention starts early)
            def load(dram, rows, cols, tag):
                tiles = []
                for i in range(rows // P):
                    t = persist.tile([P, cols], BF16, tag=f"{tag}{i}")
                    dma_copy(nc.gpsimd, t[:, :], dram[i * P:(i + 1) * P, :])
                    tiles.append(t)
                return tiles

            wkb = load(wkt, C, CH, "wk")
            wqb = load(wqt, C, CH, "wq")
            # x in query-chunk-major order so the first K/Q column chunks can
            # start as soon as possible
            xtb = [persist.tile([P, NK], BF16, tag=f"xt{i}", name=f"xt{i}")
                   for i in range(CT)]
            def load_x(qch):
                for i in range(CT):
                    dma_copy(nc.gpsimd, xtb[i][:, qch * 512:(qch + 1) * 512],
                             xt[i * P:(i + 1) * P, qch * 512:(qch + 1) * 512])
            load_x(0)
            load_x(1)
            wvb = load(wvt, C, CH, "wv")
            load_x(2)
            load_x(3)
            wpb = load(wpt, CH, C, "wp")

            onesb = persist.tile([1, DH], BF16, tag="ones")
            nc.gpsimd.memset(onesb[:, :], 1.0)

            ktb = [persist.tile([P, NK], BF16, tag=f"kt{i}", name=f"kt{i}")
                   for i in range(PT)]
            qtb = [persist.tile([P, NQ], BF16, tag=f"qt{i}", name=f"qt{i}")
                   for i in range(PT)]
            vtb = [persist.tile([P, NH * (DH + 1)], BF16, tag=f"v{i}", name=f"v{i}")
                   for i in range(KT)]
            for i in range(KT):
                ones_cols = vtb[i][:, :].rearrange(
                    "p (h e) -> p h e", e=DH + 1)[:, :, DH:DH + 1]
                nc.gpsimd.memset(ones_cols, 1.0)
            otb = [persist.tile([P, NQ], BF16, tag=f"ot{i}", name=f"ot{i}")
                   for i in range(PT)]

            # ---- work units (emitted inline or pumped from the backlog) ----
            def kq_unit(wtiles, out_tiles, pair, qch):
                # one [128, 512] column chunk of K^T (or Q^T) for head pair
                ps = psa.tile([P, QC], F32, tag="a")
                for k in range(CT):
                    nc.tensor.matmul(
                        ps[:, 0:512],
                        lhsT=wtiles[k][:, pair * P:(pair + 1) * P],
                        rhs=xtb[k][:, qch * 512:(qch + 1) * 512],
                        start=(k == 0), stop=(k == CT - 1),
                    )
                nc.vector.tensor_copy(
                    out_tiles[pair][:, qch * 512:(qch + 1) * 512], ps[:, 0:512]
                )

            def v_unit(kt):
                # V rows for key tile kt: [128, 384], scattered into stride-65
                # columns of vtb[kt] (leaves the per-head ones column alone)
                ps = psa.tile([P, QC], F32, tag="a")
                for k in range(CT):
                    nc.tensor.matmul(
                        ps[:, 0:CH],
                        lhsT=xtb[k][:, kt * P:(kt + 1) * P],
                        rhs=wvb[k][:, :],
                        start=(k == 0), stop=(k == CT - 1),
                    )
                dst = vtb[kt][:, :].rearrange("p (h e) -> p h e", e=DH + 1)[:, :, 0:DH]
                src = ps[:, 0:CH].rearrange("p (h e) -> p h e", e=DH)
                nc.vector.tensor_copy(dst, src)

            def proj_unit(qc, qt):
                # partial projection rows [128, 768] in two free-dim chunks
                q0 = qc * QC + qt * P
                yt = ysb.tile([P, C], F32, tag="y")
                for c0, csz in ((0, 512), (512, C - 512)):
                    ps = psa.tile([P, QC], F32, tag="a")
                    for k in range(PT):
                        nc.tensor.matmul(
                            ps[:, 0:csz],
                            lhsT=otb[k][:, q0:q0 + P],
                            rhs=wpb[k][:, c0:c0 + csz],
                            start=(k == 0), stop=(k == PT - 1),
                        )
                    nc.vector.tensor_copy(yt[:, c0:c0 + csz], ps[:, 0:csz])
                nc.sync.dma_start(out=y[q0:q0 + P, :], in_=yt[:, :])

            backlog = deque()

            def pump(n):
                for _ in range(min(n, len(backlog))):
                    backlog.popleft()()

            # Minimal prelude: just enough K/Q/V for (h0, qc0) to start; the
            # rest feeds the attention-loop backlog in dependency order.
            kq_unit(wkb, ktb, 0, 0)
            kq_unit(wqb, qtb, 0, 0)
            kq_unit(wqb, qtb, 0, 1)
            v_unit(0)
            v_unit(1)
            for qch in (1, 2, 3):
                backlog.append(lambda q=qch: kq_unit(wkb, ktb, 0, q))
            for kt in range(2, KT):
                backlog.append(lambda kt=kt: v_unit(kt))
            for pair in (1, 2):
                for qch in range(4):
                    backlog.append(
                        lambda p=pair, q=qch: kq_unit(wkb, ktb, p, q))
                for qch in range(4):
                    backlog.append(
                        lambda p=pair, q=qch: kq_unit(wqb, qtb, p, q))
            backlog.append(lambda: kq_unit(wqb, qtb, 0, 2))
            backlog.append(lambda: kq_unit(wqb, qtb, 0, 3))

            # ---- attention ----
            def attention(h, qc, pump_every=3, norm_slot=None):
                pair, hr = h // 2, (h % 2) * DH
                ot = psb.tile([DH + 1, QC], F32, tag="b")

                def av(pt, kt):
                    for j in range(2):
                        nc.tensor.matmul(
                            ot[:, j * 512:(j + 1) * 512],
                            lhsT=vtb[kt][:, h * (DH + 1):(h + 1) * (DH + 1)],
                            rhs=pt[:, j * 512:(j + 1) * 512],
                            start=(kt == 0), stop=(kt == KT - 1),
                        )

                pending = None
                for kt in range(KT):
                    st = psa.tile([P, QC], F32, tag="a")
                    for j in range(2):
                        nc.tensor.matmul(
                            st[:, j * 512:(j + 1) * 512],
                            lhsT=ktb[pair][hr:hr + DH, kt * P:(kt + 1) * P],
                            rhs=qtb[pair][hr:hr + DH,
                                          qc * QC + j * 512:qc * QC + (j + 1) * 512],
                            start=True, stop=True,
                            tile_position=(hr, 0),
                        )
                    if kt == 6 and norm_slot is not None:
                        norm_slot()
                    elif kt % pump_every == 0:
                        pump(1)
                    if pending is not None:
                        av(*pending)
                    pt = pP.tile([P, QC], BF16, tag="p")
                    nc.scalar.activation(
                        pt[:, :], st[:, :],
                        mybir.ActivationFunctionType.Exp, scale=SCALE,
                    )
                    pending = (pt, kt)
                pump(1)
                av(*pending)

                # stash the unnormalized output + denominators (fast PSUM
                # drain) and defer the reciprocal/broadcast/multiply into the
                # backlog so the next head's matmuls start immediately
                osb = nsb.tile([DH, QC], F32, tag="osb")
                den = nsb.tile([1, QC], F32, tag="den")
                nc.vector.tensor_copy(osb[:, :], ot[0:DH, :])
                nc.vector.tensor_copy(den[:, :], ot[DH:DH + 1, :])

                def finish_norm(pair=pair, hr=hr, qc=qc, osb=osb, den=den):
                    rec = nsb.tile([1, QC], F32, tag="rec")
                    nc.vector.reciprocal_approx_fast(out=rec[:, :], in_=den[:, :])
                    recb = nsb.tile([1, QC], BF16, tag="recb")
                    nc.vector.tensor_copy(recb[:, :], rec[:, :])
                    rb = psa.tile([P, QC], F32, tag="a")
                    for j in range(2):
                        nc.tensor.matmul(
                            rb[0:DH, j * 512:(j + 1) * 512],
                            lhsT=onesb[0:1, :],
                            rhs=recb[0:1, j * 512:(j + 1) * 512],
                            start=True, stop=True,
                        )
                    nc.vector.tensor_mul(
                        otb[pair][hr:hr + DH, qc * QC:(qc + 1) * QC],
                        osb[:, :], rb[0:DH, :],
                    )
                return finish_norm

            prev_norm = None
            for qc in range(2):
                for h in range(NH):
                    pe = 1 if (qc == 0 and h < 2) else 3
                    fn = attention(h, qc, pump_every=pe, norm_slot=prev_norm)
                    prev_norm = fn
                if qc == 0:
                    # finish the last head's norm before proj units (which
                    # read the normalized otb) enter the backlog
                    prev_norm()
                    prev_norm = None
                    for qt in range(QC // P):
                        backlog.append(lambda qt=qt: proj_unit(0, qt))
            prev_norm()
            pump(len(backlog))
            for qt in range(QC // P):
                proj_unit(1, qt)
            if debug:
                for i in range(PT):
                    tmp = ysb.tile([P, NK], F32, tag="dbg")
                    nc.vector.tensor_copy(tmp[:, :], ktb[i][:, :])
                    nc.sync.dma_start(out=dkt[i * P:(i + 1) * P, :], in_=tmp[:, :])
                    tmp = ysb.tile([P, NQ], F32, tag="dbg")
                    nc.vector.tensor_copy(tmp[:, :], qtb[i][:, :])
                    nc.sync.dma_start(out=dqt[i * P:(i + 1) * P, :], in_=tmp[:, :])
                    tmp = ysb.tile([P, NQ], F32, tag="dbg")
                    nc.vector.tensor_copy(tmp[:, :], otb[i][:, :])
                    nc.sync.dma_start(out=dot[i * P:(i + 1) * P, :], in_=tmp[:, :])
                for i in range(KT):
                    tmp = ysb.tile([P, NH * (DH + 1)], F32, tag="dbgv")
                    nc.vector.tensor_copy(tmp[:, :], vtb[i][:, :])
                    nc.sync.dma_start(out=dvt[i * P:(i + 1) * P, :], in_=tmp[:, :])

    nc.compile()
    return nc


def _get_prog() -> bass.Bass:
    global _PROG
    if _PROG is None:
        _PROG = _build_program()
    return _PROG


def kernel(x, Wq, Wk, Wv, Wp, bp):
    global LAST_RESULT
    x = np.asarray(x, np.float32)
    Wq = np.asarray(Wq, np.float32)
    Wk = np.asarray(Wk, np.float32)
    Wv = np.asarray(Wv, np.float32)
    Wp = np.asarray(Wp, np.float32)
    bp = np.asarray(bp, np.float32)

    B, N, _ = x.shape
    xts = [np.ascontiguousarray(x[b].T).astype(BF16_NP) for b in range(B)]
    wk_h, wq_h, wv_h, wp_h = [], [], [], []
    for hh in range(2):
        r = slice(hh * CH, (hh + 1) * CH)
        wq_h.append(np.ascontiguousarray(Wq[r].T).astype(BF16_NP))
        wk_h.append(np.ascontiguousarray(Wk[r].T).astype(BF16_NP))
        wv_h.append(np.ascontiguousarray(Wv[r].T).astype(BF16_NP))
        wp_h.append(np.ascontiguousarray(Wp.T[r]).astype(BF16_NP))

    in_maps = []
    for core in range(8):
        b, hh = core // 2, core % 2
        in_maps.append({
            "xt": xts[b],
            "wkt": wk_h[hh], "wqt": wq_h[hh],
            "wvt": wv_h[hh], "wpt": wp_h[hh],
        })

    res = run_bass_kernel_spmd(
        _get_prog(), in_maps, core_ids=list(range(8)),
        trace=bool(os.environ.get("BASS_TRACE")),
    )
    LAST_RESULT = res

    out = np.empty((B, N, C), np.float32)
    for b in range(B):
        out[b] = res.results[2 * b]["y"] + res.results[2 * b + 1]["y"] + bp
    return out


# revision 11
# speedup vs baseline: 1.7766x; 1.2150x over previous
"""Multi-head attention (B=4, N=2048, C=768, H=12) on 8 trn2 NeuronCores.

Sharding: core c handles batch b = c//2 and heads hh = c%2 (6 heads each:
global heads 6*hh .. 6*hh+5). Each core computes Q/K/V for its 6 heads over
all 2048 queries/keys, full attention for those heads, and a PARTIAL output
projection (contraction over its 384 channels). The host sums the two
partial projections per batch and adds the bias.

Per-core on-chip layout (all bf16 on the PE):
  xtb  = x_b.T          6 x [128, 2048]
  ktb  = Wk_h @ x.T     3 x [128, 2048]   tile t rows 64*(h%2) hold head h=2t+..
  qtb  = Wq_h @ x.T     3 x [128, 2048]
  vtb  = x @ Wv_h.T    16 x [128, 390]    per key-tile; head h cols 65h..65h+63,
                                          col 65h+64 == 1.0 (ones column makes
                                          the PV matmul also emit softmax
                                          denominators)
  st   = K_h^T Q_h      [128k, 1024q] PSUM; exp on ScalarE (scale folded)
  ot   = [V_h|1]^T P    [65, 1024] PSUM accumulated over 16 key tiles
  otb  = normalized out 3 x [128, 2048] bf16 (head-major channel layout)
  y    = otb.T @ WpT_h  [2048, 768] fp32 partial, summed on host

The scalar engine (exp: 192 x [128,1024] activations) is the pacing floor;
QKV/proj matmuls are interleaved into the attention PE stream via a backlog
queue so the PE never idles long enough to trip the HAM clock-gate.
"""

import os
import sys
from collections import deque

import numpy as np
import ml_dtypes

sys.path.insert(0, "/opt/trn_rl_repo")

import concourse.bass as bass
from concourse import bacc
import concourse.mybir as mybir
from concourse.tile import TileContext
from concourse.bass_utils import run_bass_kernel_spmd
from concourse.dma_utils import dma_copy

P = 128
C = 768
CH = 384             # channels per core (6 heads)
NK = 2048
NQ = 2048
QC = 1024            # query chunk (exp instruction free size)
NH = 6               # local heads
DH = 64
CT = C // P          # 6 contraction tiles for QKV linears
PT = CH // P         # 3 contraction tiles for proj
KT = NK // P         # 16 key tiles
SCALE = DH ** -0.5
F32 = mybir.dt.float32
BF16 = mybir.dt.bfloat16
BF16_NP = ml_dtypes.bfloat16

LAST_RESULT = None
_PROG = None


def _build_program() -> bass.Bass:
    nc = bacc.Bacc(None, target_bir_lowering=False)

    xt = nc.dram_tensor("xt", [C, NK], BF16, kind="ExternalInput")
    wkt = nc.dram_tensor("wkt", [C, CH], BF16, kind="ExternalInput")
    wqt = nc.dram_tensor("wqt", [C, CH], BF16, kind="ExternalInput")
    wvt = nc.dram_tensor("wvt", [C, CH], BF16, kind="ExternalInput")
    wpt = nc.dram_tensor("wpt", [CH, C], BF16, kind="ExternalInput")
    y = nc.dram_tensor("y", [NQ, C], F32, kind="ExternalOutput")
    debug = bool(os.environ.get("BASS_DEBUG_DUMP"))
    if debug:
        dkt = nc.dram_tensor("dkt", [CH, NK], F32, kind="ExternalOutput")
        dqt = nc.dram_tensor("dqt", [CH, NQ], F32, kind="ExternalOutput")
        dvt = nc.dram_tensor("dvt", [KT * P, NH * (DH + 1)], F32, kind="ExternalOutput")
        dot = nc.dram_tensor("dot", [CH, NQ], F32, kind="ExternalOutput")

    with TileContext(nc) as tc:
        with (
            tc.tile_pool(name="persist", bufs=1) as persist,
            tc.tile_pool(name="pP", bufs=3) as pP,
            tc.tile_pool(name="norm", bufs=2) as nsb,
            tc.tile_pool(name="ysb", bufs=2) as ysb,
            tc.tile_pool(name="psa", bufs=2, space="PSUM") as psa,
            tc.tile_pool(name="psb", bufs=2, space="PSUM") as psb,
        ):
            # ---- DMA inputs (weights for K/Q first so attention starts early)
            def load(dram, rows, cols, tag):
                tiles = []
                for i in range(rows // P):
                    t = persist.tile([P, cols], BF16, tag=f"{tag}{i}")
                    dma_copy(nc.gpsimd, t[:, :], dram[i * P:(i + 1) * P, :])
                    tiles.append(t)
                return tiles

            wkb = load(wkt, C, CH, "wk")
            wqb = load(wqt, C, CH, "wq")
            # x in query-chunk-major order so the first K/Q column chunks can
            # start as soon as possible
            xtb = [persist.tile([P, NK], BF16, tag=f"xt{i}", name=f"xt{i}")
                   for i in range(CT)]
            def load_x(qch):
                for i in range(CT):
                    dma_copy(nc.gpsimd, xtb[i][:, qch * 512:(qch + 1) * 512],
                             xt[i * P:(i + 1) * P, qch * 512:(qch + 1) * 512])
            load_x(0)
            load_x(1)
            wvb = load(wvt, C, CH, "wv")
            load_x(2)
            load_x(3)
            wpb = load(wpt, CH, C, "wp")

            onesb = persist.tile([1, DH], BF16, tag="ones")
            nc.gpsimd.memset(onesb[:, :], 1.0)

            ktb = [persist.tile([P, NK], BF16, tag=f"kt{i}", name=f"kt{i}")
                   for i in range(PT)]
            qtb = [persist.tile([P, NQ], BF16, tag=f"qt{i}", name=f"qt{i}")
                   for i in range(PT)]
            vtb = [persist.tile([P, NH * (DH + 1)], BF16, tag=f"v{i}", name=f"v{i}")
                   for i in range(KT)]
            for i in range(KT):
                ones_cols = vtb[i][:, :].rearrange(
                    "p (h e) -> p h e", e=DH + 1)[:, :, DH:DH + 1]
                nc.gpsimd.memset(ones_cols, 1.0)
            otb = [persist.tile([P, NQ], BF16, tag=f"ot{i}", name=f"ot{i}")
                   for i in range(PT)]

            # ---- work units (emitted inline or pumped from the backlog) ----
            def kq_unit(wtiles, out_tiles, pair, qch):
                # one [128, 512] column chunk of K^T (or Q^T) for head pair
                ps = psa.tile([P, QC], F32, tag="a")
                for k in range(CT):
                    nc.tensor.matmul(
                        ps[:, 0:512],
                        lhsT=wtiles[k][:, pair * P:(pair + 1) * P],
                        rhs=xtb[k][:, qch * 512:(qch + 1) * 512],
                        start=(k == 0), stop=(k == CT - 1),
                    )
                nc.vector.tensor_copy(
                    out_tiles[pair][:, qch * 512:(qch + 1) * 512], ps[:, 0:512]
                )

            def v_unit(kt):
                # V rows for key tile kt: [128, 384], scattered into stride-65
                # columns of vtb[kt] (leaves the per-head ones column alone)
                ps = psa.tile([P, QC], F32, tag="a")
                for k in range(CT):
                    nc.tensor.matmul(
                        ps[:, 0:CH],
                        lhsT=xtb[k][:, kt * P:(kt + 1) * P],
                        rhs=wvb[k][:, :],
                        start=(k == 0), stop=(k == CT - 1),
                    )
                dst = vtb[kt][:, :].rearrange("p (h e) -> p h e", e=DH + 1)[:, :, 0:DH]
                src = ps[:, 0:CH].rearrange("p (h e) -> p h e", e=DH)
                nc.vector.tensor_copy(dst, src)

            def proj_unit(qc, qt):
                # partial projection rows [128, 768] in two free-dim chunks
                q0 = qc * QC + qt * P
                yt = ysb.tile([P, C], F32, tag="y")
                for c0, csz in ((0, 512), (512, C - 512)):
                    ps = psa.tile([P, QC], F32, tag="a")
                    for k in range(PT):
                        nc.tensor.matmul(
                            ps[:, 0:csz],
                            lhsT=otb[k][:, q0:q0 + P],
                            rhs=wpb[k][:, c0:c0 + csz],
                            start=(k == 0), stop=(k == PT - 1),
                        )
                    nc.vector.tensor_copy(yt[:, c0:c0 + csz], ps[:, 0:csz])
                nc.sync.dma_start(out=y[q0:q0 + P, :], in_=yt[:, :])

            backlog = deque()

            def pump(n):
                for _ in range(min(n, len(backlog))):
                    backlog.popleft()()

            # Minimal prelude: just enough K/Q/V for (h0, qc0) to start; the
            # rest feeds the attention-loop backlog in dependency order.
            kq_unit(wkb, ktb, 0, 0)
            kq_unit(wqb, qtb, 0, 0)
            kq_unit(wqb, qtb, 0, 1)
            v_unit(0)
            v_unit(1)
            v_unit(2)
            # dependency-ordered: pumped once per kt-iteration during (qc0,h0)
            order = [("v", 3), ("kk", 0, 1), ("v", 4), ("v", 5), ("v", 6),
                     ("kk", 0, 2), ("v", 7), ("v", 8), ("v", 9), ("kk", 0, 3),
                     ("v", 10), ("v", 11), ("v", 12), ("v", 13), ("v", 14),
                     ("v", 15)]
            for u in order:
                if u[0] == "v":
                    backlog.append(lambda kt=u[1]: v_unit(kt))
                else:
                    backlog.append(lambda p=u[1], q=u[2]: kq_unit(wkb, ktb, p, q))
            for pair in (1, 2):
                for qch in range(4):
                    backlog.append(
                        lambda p=pair, q=qch: kq_unit(wkb, ktb, p, q))
                for qch in range(4):
                    backlog.append(
                        lambda p=pair, q=qch: kq_unit(wqb, qtb, p, q))
            backlog.append(lambda: kq_unit(wqb, qtb, 0, 2))
            backlog.append(lambda: kq_unit(wqb, qtb, 0, 3))

            # ---- attention ----
            def attention(h, qc, pump_every=3, norm_slot=None):
                pair, hr = h // 2, (h % 2) * DH
                ot = psb.tile([DH + 1, QC], F32, tag="b")

                def av(pt, kt):
                    for j in range(2):
                        nc.tensor.matmul(
                            ot[:, j * 512:(j + 1) * 512],
                            lhsT=vtb[kt][:, h * (DH + 1):(h + 1) * (DH + 1)],
                            rhs=pt[:, j * 512:(j + 1) * 512],
                            start=(kt == 0), stop=(kt == KT - 1),
                        )

                pending = None
                for kt in range(KT):
                    st = psa.tile([P, QC], F32, tag="a")
                    for j in range(2):
                        nc.tensor.matmul(
                            st[:, j * 512:(j + 1) * 512],
                            lhsT=ktb[pair][hr:hr + DH, kt * P:(kt + 1) * P],
                            rhs=qtb[pair][hr:hr + DH,
                                          qc * QC + j * 512:qc * QC + (j + 1) * 512],
                            start=True, stop=True,
                            tile_position=(hr, 0),
                        )
                    if kt == 6 and norm_slot is not None:
                        norm_slot()
                    elif (norm_slot is None or kt > 6) and kt % pump_every == 0:
                        pump(1)
                    if pending is not None:
                        av(*pending)
                    pt = pP.tile([P, QC], BF16, tag="p")
                    nc.scalar.activation(
                        pt[:, :], st[:, :],
                        mybir.ActivationFunctionType.Exp, scale=SCALE,
                    )
                    pending = (pt, kt)
                pump(1)
                av(*pending)

                # stash the unnormalized output + denominators (fast PSUM
                # drain) and defer the reciprocal/broadcast/multiply into the
                # backlog so the next head's matmuls start immediately
                osb = nsb.tile([DH, QC], F32, tag="osb")
                den = nsb.tile([1, QC], F32, tag="den")
                nc.vector.tensor_copy(osb[:, :], ot[0:DH, :])
                nc.vector.tensor_copy(den[:, :], ot[DH:DH + 1, :])

                def finish_norm(pair=pair, hr=hr, qc=qc, osb=osb, den=den):
                    rec = nsb.tile([1, QC], F32, tag="rec")
                    nc.vector.reciprocal_approx_fast(out=rec[:, :], in_=den[:, :])
                    recb = nsb.tile([1, QC], BF16, tag="recb")
                    nc.vector.tensor_copy(recb[:, :], rec[:, :])
                    rb = psa.tile([P, QC], F32, tag="a")
                    for j in range(2):
                        nc.tensor.matmul(
                            rb[0:DH, j * 512:(j + 1) * 512],
                            lhsT=onesb[0:1, :],
                            rhs=recb[0:1, j * 512:(j + 1) * 512],
                            start=True, stop=True,
                        )
                    nc.vector.tensor_mul(
                        otb[pair][hr:hr + DH, qc * QC:(qc + 1) * QC],
                        osb[:, :], rb[0:DH, :],
                    )
                return finish_norm

            prev_norm = None
            for qc in range(2):
                for h in range(NH):
                    pe = 1 if h < 2 else 3
                    fn = attention(h, qc, pump_every=pe, norm_slot=prev_norm)
                    prev_norm = fn
                if qc == 0:
                    # proj(qc0) units are pumped during qc1; the kt>6 gating on
                    # (qc1,h0) guarantees they follow finish_norm(h5,qc0)
                    for qt in range(QC // P):
                        backlog.append(lambda qt=qt: proj_unit(0, qt))
            prev_norm()
            pump(len(backlog))
            for qt in range(QC // P):
                proj_unit(1, qt)
            if debug:
                for i in range(PT):
                    tmp = ysb.tile([P, NK], F32, tag="dbg")
                    nc.vector.tensor_copy(tmp[:, :], ktb[i][:, :])
                    nc.sync.dma_start(out=dkt[i * P:(i + 1) * P, :], in_=tmp[:, :])
                    tmp = ysb.tile([P, NQ], F32, tag="dbg")
                    nc.vector.tensor_copy(tmp[:, :], qtb[i][:, :])
                    nc.sync.dma_start(out=dqt[i * P:(i + 1) * P, :], in_=tmp[:, :])
                    tmp = ysb.tile([P, NQ], F32, tag="dbg")
                    nc.vector.tensor_copy(tmp[:, :], otb[i][:, :])
                    nc.sync.dma_start(out=dot[i * P:(i + 1) * P, :], in_=tmp[:, :])
                for i in range(KT):
                    tmp = ysb.tile([P, NH * (DH + 1)], F32, tag="dbgv")
                    nc.vector.tensor_copy(tmp[:, :], vtb[i][:, :])
                    nc.sync.dma_start(out=dvt[i * P:(i + 1) * P, :], in_=tmp[:, :])

    nc.compile()
    return nc


def _get_prog() -> bass.Bass:
    global _PROG
    if _PROG is None:
        _PROG = _build_program()
    return _PROG


def kernel(x, Wq, Wk, Wv, Wp, bp):
    global LAST_RESULT
    x = np.asarray(x, np.float32)
    Wq = np.asarray(Wq, np.float32)
    Wk = np.asarray(Wk, np.float32)
    Wv = np.asarray(Wv, np.float32)
    Wp = np.asarray(Wp, np.float32)
    bp = np.asarray(bp, np.float32)

    B, N, _ = x.shape
    xts = [np.ascontiguousarray(x[b].T).astype(BF16_NP) for b in range(B)]
    wk_h, wq_h, wv_h, wp_h = [], [], [], []
    for hh in range(2):
        r = slice(hh * CH, (hh + 1) * CH)
        wq_h.append(np.ascontiguousarray(Wq[r].T).astype(BF16_NP))
        wk_h.append(np.ascontiguousarray(Wk[r].T).astype(BF16_NP))
        wv_h.append(np.ascontiguousarray(Wv[r].T).astype(BF16_NP))
        wp_h.append(np.ascontiguousarray(Wp.T[r]).astype(BF16_NP))

    in_maps = []
    for core in range(8):
        b, hh = core // 2, core % 2
        in_maps.append({
            "xt": xts[b],
            "wkt": wk_h[hh], "wqt": wq_h[hh],
            "wvt": wv_h[hh], "wpt": wp_h[hh],
        })

    res = run_bass_kernel_spmd(
        _get_prog(), in_maps, core_ids=list(range(8)),
        trace=bool(os.environ.get("BASS_TRACE")),
    )
    LAST_RESULT = res

    out = np.empty((B, N, C), np.float32)
    for b in range(B):
        out[b] = res.results[2 * b]["y"] + res.results[2 * b + 1]["y"] + bp
    return out
